# revision 1
# baseline (speedup 1.0000x reference)
"""PointsRenderer (alpha compositing over K points/pixel) on 8 trn2 cores.

Sharding: data-parallel over batch B=8 -> 1 image per NeuronCore; the
[100000, 4] feature table is replicated per core and gathered with
per-partition indirect DMA (128 rows per call; on this hardware the
indirect DMA consumes exactly one offset per output partition, so the
gather is FT calls per tile).

Per-core layout: the 512*512*8 = 2^21 fragment stream is split over the
128 SBUF partitions (16384 fragments each), processed in NT tiles of FT
fragments per partition.  Compositing (weights, front-to-back
transmittance cumprod, contrib) streams on DVE/ACT and overlaps the
gather; the K-sum is a tree reduction feeding a compact output tile.
"""

import numpy as np

import concourse.bass as bass
import concourse.mybir as mybir
import concourse.tile as tile
from concourse import bacc
from concourse.bass_utils import run_bass_kernel_spmd

B, H, W, K, P, C = 8, 512, 512, 8, 100000, 4
NF = H * W * K          # fragments per core (B=1 shard)
PARTS = 128
PERPART = NF // PARTS   # 16384
FT = 256                # fragments per partition per tile (32K descriptors/gather call)
NT = PERPART // FT      # 16
PIX_T = FT // K         # 128 pixels per partition per tile

F32 = mybir.dt.float32
I32 = mybir.dt.int32


def build(inv_r2: float, idx_words: int):
    """idx_words: 2 when host idx is int64 (little-endian pairs), 1 for int32."""
    nc = bacc.Bacc(None, target_bir_lowering=False, debug=False)
    idx32 = nc.dram_tensor(
        "idx32", [PARTS, NT, FT * idx_words], I32, kind="ExternalInput"
    )
    d2 = nc.dram_tensor("d2", [PARTS, NT, FT], F32, kind="ExternalInput")
    feat = nc.dram_tensor("feat", [P, C], F32, kind="ExternalInput")
    out = nc.dram_tensor("out", [PARTS, NT, PIX_T * C], F32, kind="ExternalOutput")

    with tile.TileContext(nc) as tc:
        with tc.tile_pool(name="io", bufs=2) as io, \
             tc.tile_pool(name="gp", bufs=2) as gp, \
             tc.tile_pool(name="wp", bufs=2) as wp:
            for t in range(NT):
                ipair = io.tile([PARTS, FT * idx_words], I32, tag="ipair")
                nc.sync.dma_start(ipair[:], idx32[:, t, :])
                d2t = io.tile([PARTS, FT], F32, tag="d2t")
                nc.sync.dma_start(d2t[:], d2[:, t, :])

                if idx_words == 2:
                    # extract low 32-bit words of the little-endian int64 indices
                    ilow = io.tile([PARTS, FT], I32, tag="ilow")
                    nc.vector.tensor_copy(
                        ilow[:],
                        ipair[:].rearrange("p (f two) -> p f two", two=2)[:, :, 0],
                    )
                else:
                    ilow = ipair

                # gather feature rows: G[p, f*C:(f+1)*C] = feat[ilow[p, f], :]
                # HW indirect DMA consumes exactly one offset per output
                # partition, so issue FT calls of 128 rows each.
                G = gp.tile([PARTS, FT * C], F32, tag="G")
                for f in range(FT):
                    nc.gpsimd.indirect_dma_start(
                        out=G[:, f * C:(f + 1) * C],
                        out_offset=None,
                        in_=feat[:],
                        in_offset=bass.IndirectOffsetOnAxis(
                            ap=ilow[:, f:f + 1], axis=0
                        ),
                    )

                # alpha_k = 1 - d2*inv_r2 (ACT), om_k = d2*inv_r2 (DVE)
                alpha = wp.tile([PARTS, FT], F32, tag="alpha")
                nc.scalar.activation(
                    alpha[:], d2t[:], mybir.ActivationFunctionType.Copy,
                    bias=1.0, scale=-float(inv_r2),
                )
                om = wp.tile([PARTS, FT], F32, tag="om")
                nc.vector.tensor_scalar_mul(om[:], d2t[:], float(inv_r2))

                # contrib_k = alpha_k * prod_{j<k} om_j   (front-to-back)
                cb = wp.tile([PARTS, FT], F32, tag="cb")
                cbv = cb[:].rearrange("p (t k) -> p t k", k=K)
                alv = alpha[:].rearrange("p (t k) -> p t k", k=K)
                omv = om[:].rearrange("p (t k) -> p t k", k=K)
                rt = wp.tile([PARTS, PIX_T], F32, tag="rt")
                nc.vector.tensor_copy(cbv[:, :, 0], alv[:, :, 0])
                nc.vector.tensor_copy(rt[:], omv[:, :, 0])
                for k in range(1, K):
                    nc.vector.tensor_mul(cbv[:, :, k], alv[:, :, k], rt[:])
                    if k < K - 1:
                        nc.vector.tensor_mul(rt[:], rt[:], omv[:, :, k])

                # G *= contrib (broadcast over channel)
                G3 = G[:].rearrange("p (f c) -> p f c", c=C)
                nc.vector.tensor_mul(
                    G3, G3,
                    cb[:].rearrange("p (f one) -> p f one", one=1).to_broadcast([PARTS, FT, C]),
                )

                # sum over K: tree reduction, final into compact tile
                Gv = G[:].rearrange("p (t k c) -> p t k c", k=K, c=C)
                nc.vector.tensor_add(Gv[:, :, 0:4, :], Gv[:, :, 0:4, :], Gv[:, :, 4:8, :])
                nc.vector.tensor_add(Gv[:, :, 0:2, :], Gv[:, :, 0:2, :], Gv[:, :, 2:4, :])
                outT = wp.tile([PARTS, PIX_T, C], F32, tag="outT")
                nc.vector.tensor_add(outT[:], Gv[:, :, 0, :], Gv[:, :, 1, :])

                nc.sync.dma_start(out[:, t, :], outT[:].rearrange("p t c -> p (t c)"))

    nc.compile()
    return nc


last_result = None
last_nc = None
last_in_maps = None


def kernel(idx, dists2, features, radius):
    global last_result, last_nc, last_in_maps
    idx = np.ascontiguousarray(idx)
    dists2 = np.ascontiguousarray(dists2, dtype=np.float32)
    features = np.ascontiguousarray(features, dtype=np.float32)
    r = float(np.asarray(radius).reshape(-1)[0])
    inv_r2 = 1.0 / (r * r)

    if idx.dtype == np.int64:
        idx_words = 2
    else:
        idx = np.ascontiguousarray(idx, dtype=np.int32)
        idx_words = 1

    nc = build(inv_r2, idx_words)

    in_maps = []
    for b in range(B):
        idx32_b = idx[b].reshape(-1).view(np.int32).reshape(PARTS, NT, FT * idx_words)
        d2_b = dists2[b].reshape(PARTS, NT, FT)
        in_maps.append({"idx32": idx32_b, "d2": d2_b, "feat": features})

    last_nc, last_in_maps = nc, in_maps
    res = run_bass_kernel_spmd(nc, in_maps, core_ids=list(range(B)))
    last_result = res

    out = np.empty((B, H, W, C), dtype=np.float32)
    for b in range(B):
        out[b] = res.results[b]["out"].reshape(H, W, C)
    return out

